# revision 20
# baseline (speedup 1.0000x reference)
"""Additive (Bahdanau) attention kernel for 8 Trainium2 NeuronCores.

Reference computation (per (b,h) block, Lq=Lk=256, dk=64):
    qp = q @ Wq_w.T + Wq_b
    kp = k @ Wk_w.T + Wk_b
    scores[q,k] = vs_w . tanh(qp[q,:] + kp[k,:]) + vs_b
    attn = softmax(scores, axis=k)
    out  = attn @ v
Returns (out, attn).

Key observations:
  * vs_b is constant along k, so softmax cancels it exactly -> dropped.
  * |scores| <= sum|vs_w| <= 8, so exp() needs no max-subtraction.
  * B*H = 32 independent blocks -> 4 per core, params replicated, no
    collectives.

Device layout (per block): partitions = 128 = 2 x dk, packing the query
pair (2j, 2j+1) per tile column; k runs along the free dim.
  - kp2 [128, 256]  : kp^T duplicated in both partition halves
  - qp2 [128, 128]  : column j = [qp^T[:,2j] ; qp^T[:,2j+1]]
  - DVE tensor_scalar add builds pre[:, j-slice] = kp2 + qp2[:,j]
  - ACT does one big tanh per batch of pairs (the DVE adds are the
    kernel's critical path: DVE fast modes don't engage on this build,
    so everything is sized around 1x DVE streaming)
  - PE reduces over dk with stationary=tanh tile (bf16, FWL) and
    moving=vs2 [128,2] block-diagonal vs_w -> scoresT [k, q] in PSUM,
    one PSUM tile per q-half so softmax can drain early
  - softmax over k (partitions) via ones-matmul column sums, chunked by
    q-half to shorten the kernel tail
  - out = attnT.T @ v needs no transpose since attnT already has k on
    partitions; the host transposes attnT -> attn at the end.
"""

import os
from contextlib import ExitStack

import numpy as np

os.environ.setdefault("MYCRO_LOCAL_CACHE", "1")

import concourse.bass as bass
import concourse.bacc as bacc
import concourse.tile as tile
from concourse import mybir
from concourse.bass_utils import run_bass_kernel_spmd

F32 = mybir.dt.float32
BF16 = mybir.dt.bfloat16
AF = mybir.ActivationFunctionType

B, H, LQ, LK, DK = 4, 8, 256, 256, 64
NCORES = 8
NBLK = (B * H) // NCORES  # blocks per core
NPAIR = LQ // 2  # q-pairs per block; pair j = queries (2j, 2j+1)
PAIRS_PER_HALF = NPAIR // 2  # pairs per q-half (queries 0:128 / 128:256)


def batches_for_block(b, nblk):
    if b < nblk - 1:
        return [16] * 8
    # taper the final block so the ACT pipeline drains quickly
    return [16] * 6 + [8] * 4


def build_nc(nblk=NBLK):
    nc = bacc.Bacc(None, target_bir_lowering=False)
    # qkT[b]: [128, 256] = [qT (d x Lq) ; kT (d x Lk)] stacked on partitions
    qkT_d = nc.declare_dram_parameter("qkT", [nblk, 2 * DK, LQ], F32, isOutput=False)
    v_d = nc.declare_dram_parameter("v", [nblk, LK, DK], F32, isOutput=False)
    # par[0:64, 0:64]=WqT, par[64:128, 0:64]=WkT (each projection matmul then
    # has lhsT/rhs at the same base partition); par[0:64, 64]=Wqb,
    # par[0:64, 65]=Wkb, par[0:64, 66]=vs
    par_d = nc.declare_dram_parameter("par", [2 * DK, DK + 3], F32, isOutput=False)
    out_d = nc.declare_dram_parameter("out", [nblk, LQ, DK], F32, isOutput=True)
    attnT_d = nc.declare_dram_parameter("attnT", [nblk, LK, LQ], F32, isOutput=True)

    with ExitStack() as ctx:
        tc = ctx.enter_context(tile.TileContext(nc))
        consts = ctx.enter_context(tc.tile_pool(name="consts", bufs=1))
        proj_in = ctx.enter_context(tc.tile_pool(name="proj_in", bufs=2))
        blk = ctx.enter_context(tc.tile_pool(name="blk", bufs=2))
        pre_pool = ctx.enter_context(tc.tile_pool(name="pre", bufs=3))
        tanh_pool = ctx.enter_context(tc.tile_pool(name="tanh", bufs=3))
        soft = ctx.enter_context(tc.tile_pool(name="soft", bufs=2))
        ps_proj = ctx.enter_context(tc.tile_pool(name="ps_proj", bufs=2, space="PSUM"))
        ps_scores = ctx.enter_context(
            tc.tile_pool(name="ps_scores", bufs=2, space="PSUM")
        )
        ps_small = ctx.enter_context(tc.tile_pool(name="ps_small", bufs=2, space="PSUM"))

        # ---- constants (single DMA) ----
        par_sb = consts.tile([2 * DK, DK + 3], F32)
        nc.sync.dma_start(out=par_sb, in_=par_d[:, :])
        WqT_sb = par_sb[0:DK, 0:DK]
        WkT_sb = par_sb[DK : 2 * DK, 0:DK]
        Wqb_sb = par_sb[0:DK, DK : DK + 1]
        Wkb_sb = par_sb[0:DK, DK + 1 : DK + 2]
        vsc_sb = par_sb[0:DK, DK + 2 : DK + 3]

        # vs2 [128, 2] block diagonal: col0 = [vs;0], col1 = [0;vs]
        vs2 = consts.tile([128, 2], BF16)
        nc.vector.memset(vs2, 0.0)
        nc.vector.tensor_copy(vs2[0:DK, 0:1], vsc_sb)
        nc.vector.tensor_copy(vs2[DK : 2 * DK, 1:2], vsc_sb)

        ones_m = consts.tile([1, 128], BF16)  # lhsT for recip broadcast (K=1)
        nc.vector.memset(ones_m, 1.0)
        ones_k = consts.tile([128, 1], BF16)  # lhsT for column sums (M=1)
        nc.vector.memset(ones_k, 1.0)

        state = {}

        def prologue(b):
            """DMA + projections + pair packing for block b."""
            qkT_sb = proj_in.tile([2 * DK, LQ], F32, tag="qkT_sb")
            nc.sync.dma_start(out=qkT_sb, in_=qkT_d[b])
            v_sb = proj_in.tile([128, 2, DK], F32, tag="v_sb")
            nc.sync.dma_start(out=v_sb, in_=v_d[b].rearrange("(h p) d -> p h d", p=128))

            ps_qp = ps_proj.tile([DK, LQ], F32, tag="ps_proj")
            nc.tensor.matmul(
                ps_qp, lhsT=WqT_sb, rhs=qkT_sb[0:DK, :], start=True, stop=True
            )
            qp2 = blk.tile([128, NPAIR], F32, tag="qp2")
            nc.vector.tensor_scalar_add(qp2[0:DK, :], ps_qp[:, 0:LQ:2], Wqb_sb)
            nc.vector.tensor_scalar_add(qp2[DK : 2 * DK, :], ps_qp[:, 1:LQ:2], Wqb_sb)

            ps_kp = ps_proj.tile([DK, LK], F32, tag="ps_proj")
            nc.tensor.matmul(
                ps_kp, lhsT=WkT_sb, rhs=qkT_sb[DK : 2 * DK, :], start=True, stop=True
            )
            kp2 = blk.tile([128, LK], BF16, tag="kp2")
            nc.vector.tensor_scalar_add(kp2[0:DK, :], ps_kp, Wkb_sb)
            nc.vector.tensor_scalar_add(kp2[DK : 2 * DK, :], ps_kp, Wkb_sb)
            state[b] = (qp2, kp2, v_sb)

        def main_loop(b):
            """DVE adds + ACT tanh + PE score reductions for block b."""
            qp2, kp2, v_sb = state[b]
            # scoresT psum per q-half: cols = h*128 + q_local
            ps_a = ps_scores.tile([128, LQ], F32, tag="ps_a")
            ps_b = ps_scores.tile([128, LQ], F32, tag="ps_b")

            j = 0
            for nb in batches_for_block(b, nblk):
                pre = pre_pool.tile([128, 16 * LK], BF16, tag="pre")
                th = tanh_pool.tile([128, 16 * LK], BF16, tag="tanh")
                for jj in range(nb):
                    nc.vector.tensor_scalar_add(
                        pre[:, jj * LK : (jj + 1) * LK], kp2, qp2[:, j + jj : j + jj + 1]
                    )
                nc.scalar.activation(
                    th[:, 0 : nb * LK], pre[:, 0 : nb * LK], AF.Tanh
                )
                for jj in range(nb):
                    jt = j + jj
                    half, jl = divmod(jt, PAIRS_PER_HALF)
                    ps_half = ps_a if half == 0 else ps_b
                    for h in range(2):
                        nc.tensor.matmul(
                            ps_half[:, h * 128 + 2 * jl : h * 128 + 2 * jl + 2],
                            lhsT=th[:, jj * LK + h * 128 : jj * LK + (h + 1) * 128],
                            rhs=vs2,
                            start=True,
                            stop=True,
                        )
                j += nb
            state[b] = (qp2, kp2, v_sb, ps_a, ps_b)

        def softmax_half(b, half, attnT_f):
            """exp -> sums -> normalize -> attn@v for one q-half of block b."""
            _, _, v_sb, ps_a, ps_b = state[b]
            ps_half = ps_a if half == 0 else ps_b
            exp_sb = soft.tile([128, LQ], BF16, tag="exp_sb")
            nc.scalar.activation(exp_sb, ps_half, AF.Exp)
            ps_sum = ps_small.tile([1, 128], F32, tag="ps_small")
            for h in range(2):
                nc.tensor.matmul(
                    ps_sum,
                    lhsT=ones_k,
                    rhs=exp_sb[:, h * 128 : (h + 1) * 128],
                    start=(h == 0),
                    stop=(h == 1),
                )
            recip_f = soft.tile([1, 128], F32, tag="recip_f")
            nc.vector.reciprocal(recip_f, ps_sum)
            recip_bf = soft.tile([1, 128], BF16, tag="recip_bf")
            nc.vector.tensor_copy(recip_bf, recip_f)
            ps_bc = ps_small.tile([128, 128], F32, tag="ps_small")
            nc.tensor.matmul(ps_bc, lhsT=ones_m, rhs=recip_bf, start=True, stop=True)
            recipB = soft.tile([128, 128], BF16, tag="recipB")
            nc.vector.tensor_copy(recipB, ps_bc)

            for h in range(2):
                nc.vector.tensor_mul(
                    attnT_f[:, h, half * 128 : (half + 1) * 128],
                    exp_sb[:, h * 128 : (h + 1) * 128],
                    recipB,
                )
            nc.sync.dma_start(
                out=attnT_d[b]
                .rearrange("(h p) q -> p h q", p=128)[:, :, half * 128 : (half + 1) * 128],
                in_=attnT_f[:, :, half * 128 : (half + 1) * 128],
            )
            # out[q, d] for this q-half
            ps_o = ps_small.tile([128, DK], F32, tag="ps_small")
            for h in range(2):
                nc.tensor.matmul(
                    ps_o,
                    lhsT=attnT_f[:, h, half * 128 : (half + 1) * 128],
                    rhs=v_sb[:, h, :],
                    start=(h == 0),
                    stop=(h == 1),
                )
            o_sb = soft.tile([128, DK], F32, tag="o_sb")
            nc.vector.tensor_copy(o_sb, ps_o)
            nc.sync.dma_start(
                out=out_d[b, half * 128 : (half + 1) * 128, :], in_=o_sb
            )

        prologue(0)
        for b in range(nblk):
            main_loop(b)
            if b + 1 < nblk:
                prologue(b + 1)
            attnT_f = soft.tile([128, 2, LQ], F32, tag="attnT_f")
            for half in range(2):
                softmax_half(b, half, attnT_f)
            del state[b]

    return nc


_CACHED = {}


def _get_nc(nblk=NBLK):
    if nblk not in _CACHED:
        nc = build_nc(nblk)
        nc.finalize()
        _CACHED[nblk] = nc
    return _CACHED[nblk]


def _prep_in_maps(q, k, v, Wq_w, Wq_b, Wk_w, Wk_b, vs_w):
    q3 = np.asarray(q, np.float32).reshape(B * H, LQ, DK)
    k3 = np.asarray(k, np.float32).reshape(B * H, LK, DK)
    v3 = np.asarray(v, np.float32).reshape(B * H, LK, DK)
    qkT = np.concatenate(
        [q3.transpose(0, 2, 1), k3.transpose(0, 2, 1)], axis=1
    )  # [B*H, 128, 256]
    par = np.zeros((2 * DK, DK + 3), np.float32)
    par[0:DK, 0:DK] = np.asarray(Wq_w, np.float32).T
    par[DK : 2 * DK, 0:DK] = np.asarray(Wk_w, np.float32).T
    par[0:DK, DK] = np.asarray(Wq_b, np.float32).reshape(DK)
    par[0:DK, DK + 1] = np.asarray(Wk_b, np.float32).reshape(DK)
    par[0:DK, DK + 2] = np.asarray(vs_w, np.float32).reshape(DK)
    in_maps = []
    for i in range(NCORES):
        s = slice(i * NBLK, (i + 1) * NBLK)
        in_maps.append(
            {
                "qkT": np.ascontiguousarray(qkT[s]),
                "v": np.ascontiguousarray(v3[s]),
                "par": par,
            }
        )
    return in_maps


def _run(inputs, trace=False):
    nc = _get_nc()
    in_maps = _prep_in_maps(
        inputs["q"], inputs["k"], inputs["v"], inputs["Wq_w"], inputs["Wq_b"],
        inputs["Wk_w"], inputs["Wk_b"], inputs["vs_w"],
    )
    res = run_bass_kernel_spmd(nc, in_maps, list(range(NCORES)), trace=trace)
    outs = np.stack([res.results[i]["out"] for i in range(NCORES)])
    attnTs = np.stack([res.results[i]["attnT"] for i in range(NCORES)])
    output = outs.reshape(B, H, LQ, DK)
    attn = attnTs.reshape(B * H, LK, LQ).transpose(0, 2, 1).reshape(B, H, LQ, LK)
    return (output, np.ascontiguousarray(attn)), res


def kernel(q, k, v, Wq_w, Wq_b, Wk_w, Wk_b, vs_w, vs_b):
    (output, attn), _ = _run(
        {
            "q": q, "k": k, "v": v, "Wq_w": Wq_w, "Wq_b": Wq_b,
            "Wk_w": Wk_w, "Wk_b": Wk_b, "vs_w": vs_w,
        }
    )
    return output, attn


# revision 26
# speedup vs baseline: 1.0258x; 1.0258x over previous
"""Additive (Bahdanau) attention kernel for 8 Trainium2 NeuronCores.

Reference computation (per (b,h) block, Lq=Lk=256, dk=64):
    qp = q @ Wq_w.T + Wq_b
    kp = k @ Wk_w.T + Wk_b
    scores[q,k] = vs_w . tanh(qp[q,:] + kp[k,:]) + vs_b
    attn = softmax(scores, axis=k)
    out  = attn @ v
Returns (out, attn).

Key observations:
  * vs_b is constant along k, so softmax cancels it exactly -> dropped.
  * |scores| <= sum|vs_w| <= 8, so exp() needs no max-subtraction.
  * B*H = 32 independent blocks -> 4 per core, params replicated, no
    collectives.

Device layout (per block): partitions = 128 = 2 x dk, packing the query
pair (2j, 2j+1) per tile column; k runs along the free dim.
  - kp2 [128, 256]  : kp^T duplicated in both partition halves
  - qp2 [128, 128]  : column j = [qp^T[:,2j] ; qp^T[:,2j+1]]
  - DVE tensor_scalar add builds pre[:, j-slice] = kp2 + qp2[:,j]
  - ACT does one big tanh per batch of pairs (the DVE adds are the
    kernel's critical path: DVE fast modes don't engage on this build,
    so everything is sized around 1x DVE streaming)
  - PE reduces over dk with stationary=tanh tile (bf16, FWL) and
    moving=vs2 [128,2] block-diagonal vs_w -> scoresT [k, q] in PSUM,
    one PSUM tile per q-half so softmax can drain early
  - softmax over k (partitions) via ones-matmul column sums, chunked by
    q-half to shorten the kernel tail
  - out = attnT.T @ v needs no transpose since attnT already has k on
    partitions; the host transposes attnT -> attn at the end.
"""

import os
from contextlib import ExitStack

import numpy as np

os.environ.setdefault("MYCRO_LOCAL_CACHE", "1")

import concourse.bass as bass
import concourse.bacc as bacc
import concourse.tile as tile
from concourse import mybir
from concourse.bass_utils import run_bass_kernel_spmd

F32 = mybir.dt.float32
BF16 = mybir.dt.bfloat16
AF = mybir.ActivationFunctionType

B, H, LQ, LK, DK = 4, 8, 256, 256, 64
NCORES = 8
NBLK = (B * H) // NCORES  # blocks per core
NPAIR = LQ // 2  # q-pairs per block; pair j = queries (2j, 2j+1)
PAIRS_PER_HALF = NPAIR // 2  # pairs per q-half (queries 0:128 / 128:256)


def batches_for_block(b, nblk):
    if b < nblk - 1:
        return [16] * 8
    # taper the final block so the ACT pipeline drains quickly
    return [16] * 6 + [8] * 4


def build_nc(nblk=NBLK):
    nc = bacc.Bacc(None, target_bir_lowering=False)
    # qkT[b]: [128, 256] = [qT (d x Lq) ; kT (d x Lk)] stacked on partitions
    qkT_d = nc.declare_dram_parameter("qkT", [nblk, 2 * DK, LQ], F32, isOutput=False)
    v_d = nc.declare_dram_parameter("v", [nblk, LK, DK], F32, isOutput=False)
    # par[0:64, 0:64]=WqT, par[64:128, 0:64]=WkT (each projection matmul then
    # has lhsT/rhs at the same base partition); par[0:64, 64]=Wqb,
    # par[0:64, 65]=Wkb, par[0:64, 66]=vs
    par_d = nc.declare_dram_parameter("par", [2 * DK, DK + 3], F32, isOutput=False)
    out_d = nc.declare_dram_parameter("out", [nblk, LQ, DK], F32, isOutput=True)
    # attnP[b, p, half, h, ql] = attn^T[b, k=h*128+p, q=half*128+ql] — laid out
    # so each q-half DMA writes 1KB contiguous per partition; host unscrambles.
    attnP_d = nc.declare_dram_parameter(
        "attnP", [nblk, 128, 2, 2, 128], F32, isOutput=True
    )

    with ExitStack() as ctx:
        tc = ctx.enter_context(tile.TileContext(nc))
        consts = ctx.enter_context(tc.tile_pool(name="consts", bufs=1))
        proj_in = ctx.enter_context(tc.tile_pool(name="proj_in", bufs=2))
        blk = ctx.enter_context(tc.tile_pool(name="blk", bufs=2))
        pre_pool = ctx.enter_context(tc.tile_pool(name="pre", bufs=3))
        tanh_pool = ctx.enter_context(tc.tile_pool(name="tanh", bufs=3))
        soft = ctx.enter_context(tc.tile_pool(name="soft", bufs=2))
        ps_proj = ctx.enter_context(tc.tile_pool(name="ps_proj", bufs=2, space="PSUM"))
        ps_scores = ctx.enter_context(
            tc.tile_pool(name="ps_scores", bufs=2, space="PSUM")
        )
        ps_small = ctx.enter_context(tc.tile_pool(name="ps_small", bufs=2, space="PSUM"))

        # ---- constants (single DMA) ----
        par_sb = consts.tile([2 * DK, DK + 3], F32)
        nc.sync.dma_start(out=par_sb, in_=par_d[:, :])
        WqT_sb = par_sb[0:DK, 0:DK]
        WkT_sb = par_sb[DK : 2 * DK, 0:DK]
        Wqb_sb = par_sb[0:DK, DK : DK + 1]
        Wkb_sb = par_sb[0:DK, DK + 1 : DK + 2]
        vsc_sb = par_sb[0:DK, DK + 2 : DK + 3]

        # vs2 [128, 2] block diagonal: col0 = [vs;0], col1 = [0;vs]
        vs2 = consts.tile([128, 2], BF16)
        nc.vector.memset(vs2, 0.0)
        nc.vector.tensor_copy(vs2[0:DK, 0:1], vsc_sb)
        nc.vector.tensor_copy(vs2[DK : 2 * DK, 1:2], vsc_sb)

        ones_m = consts.tile([1, 128], BF16)  # lhsT for recip broadcast (K=1)
        nc.vector.memset(ones_m, 1.0)
        ones_k = consts.tile([128, 1], BF16)  # lhsT for column sums (M=1)
        nc.vector.memset(ones_k, 1.0)

        state = {}

        def prologue(b):
            """DMA + projections + pair packing for block b. Input DMAs ride
            the ACT HWDGE queue so they don't contend with output DMAs."""
            qkT_sb = proj_in.tile([2 * DK, LQ], F32, tag="qkT_sb")
            nc.scalar.dma_start(out=qkT_sb, in_=qkT_d[b])
            v_sb = proj_in.tile([128, 2, DK], F32, tag="v_sb")
            nc.scalar.dma_start(
                out=v_sb, in_=v_d[b].rearrange("(h p) d -> p h d", p=128)
            )

            ps_qp = ps_proj.tile([DK, LQ], F32, tag="ps_proj")
            nc.tensor.matmul(
                ps_qp, lhsT=WqT_sb, rhs=qkT_sb[0:DK, :], start=True, stop=True
            )
            qp2 = blk.tile([128, NPAIR], F32, tag="qp2")
            nc.vector.tensor_scalar_add(qp2[0:DK, :], ps_qp[:, 0:LQ:2], Wqb_sb)
            nc.vector.tensor_scalar_add(qp2[DK : 2 * DK, :], ps_qp[:, 1:LQ:2], Wqb_sb)

            ps_kp = ps_proj.tile([DK, LK], F32, tag="ps_proj")
            nc.tensor.matmul(
                ps_kp, lhsT=WkT_sb, rhs=qkT_sb[DK : 2 * DK, :], start=True, stop=True
            )
            kp2 = blk.tile([128, LK], BF16, tag="kp2")
            nc.vector.tensor_scalar_add(kp2[0:DK, :], ps_kp, Wkb_sb)
            nc.vector.tensor_scalar_add(kp2[DK : 2 * DK, :], ps_kp, Wkb_sb)
            state[b] = (qp2, kp2, v_sb)

        def main_loop(b):
            """DVE adds + ACT tanh + PE score reductions for block b."""
            qp2, kp2, v_sb = state[b]
            # scoresT psum per q-half: cols = h*128 + q_local
            ps_a = ps_scores.tile([128, LQ], F32, tag="ps_a")
            ps_b = ps_scores.tile([128, LQ], F32, tag="ps_b")

            state[b] = (qp2, kp2, v_sb, ps_a, ps_b)
            attnT_f = soft.tile([128, 2, 2, 128], F32, tag="attnT_f")

            j = 0
            for nb in batches_for_block(b, nblk):
                pre = pre_pool.tile([128, 16 * LK], BF16, tag="pre")
                th = tanh_pool.tile([128, 16 * LK], BF16, tag="tanh")
                for jj in range(nb):
                    nc.vector.tensor_scalar_add(
                        pre[:, jj * LK : (jj + 1) * LK], kp2, qp2[:, j + jj : j + jj + 1]
                    )
                nc.scalar.activation(
                    th[:, 0 : nb * LK], pre[:, 0 : nb * LK], AF.Tanh
                )
                for jj in range(nb):
                    jt = j + jj
                    half, jl = divmod(jt, PAIRS_PER_HALF)
                    ps_half = ps_a if half == 0 else ps_b
                    for h in range(2):
                        nc.tensor.matmul(
                            ps_half[:, h * 128 + 2 * jl : h * 128 + 2 * jl + 2],
                            lhsT=th[:, jj * LK + h * 128 : jj * LK + (h + 1) * 128],
                            rhs=vs2,
                            start=True,
                            stop=True,
                        )
                j += nb
                if j == PAIRS_PER_HALF:
                    # q-half 0 is complete: drain its softmax now so only
                    # half 1's chain sits on the kernel tail
                    softmax_half(b, 0, attnT_f)
            softmax_half(b, 1, attnT_f)

        def softmax_half(b, half, attnT_f):
            """exp -> sums -> normalize -> attn@v for one q-half of block b."""
            _, _, v_sb, ps_a, ps_b = state[b]
            ps_half = ps_a if half == 0 else ps_b
            exp_sb = soft.tile([128, LQ], BF16, tag="exp_sb")
            nc.scalar.activation(exp_sb, ps_half, AF.Exp)
            ps_sum = ps_small.tile([1, 128], F32, tag="ps_small")
            for h in range(2):
                nc.tensor.matmul(
                    ps_sum,
                    lhsT=ones_k,
                    rhs=exp_sb[:, h * 128 : (h + 1) * 128],
                    start=(h == 0),
                    stop=(h == 1),
                )
            recip_f = soft.tile([1, 128], F32, tag="recip_f")
            nc.vector.reciprocal(recip_f, ps_sum)
            recip_bf = soft.tile([1, 128], BF16, tag="recip_bf")
            nc.vector.tensor_copy(recip_bf, recip_f)
            ps_bc = ps_small.tile([128, 128], F32, tag="ps_small")
            nc.tensor.matmul(ps_bc, lhsT=ones_m, rhs=recip_bf, start=True, stop=True)
            recipB = soft.tile([128, 128], BF16, tag="recipB")
            nc.vector.tensor_copy(recipB, ps_bc)

            for h in range(2):
                nc.vector.tensor_mul(
                    attnT_f[:, half, h, :],
                    exp_sb[:, h * 128 : (h + 1) * 128],
                    recipB,
                )
            nc.sync.dma_start(out=attnP_d[b, :, half], in_=attnT_f[:, half])
            # out[q, d] for this q-half
            ps_o = ps_small.tile([128, DK], F32, tag="ps_small")
            for h in range(2):
                nc.tensor.matmul(
                    ps_o,
                    lhsT=attnT_f[:, half, h, :],
                    rhs=v_sb[:, h, :],
                    start=(h == 0),
                    stop=(h == 1),
                )
            o_sb = soft.tile([128, DK], F32, tag="o_sb")
            nc.vector.tensor_copy(o_sb, ps_o)
            nc.sync.dma_start(
                out=out_d[b, half * 128 : (half + 1) * 128, :], in_=o_sb
            )

        prologue(0)
        for b in range(nblk):
            main_loop(b)
            if b + 1 < nblk:
                prologue(b + 1)
            del state[b]

    return nc


_CACHED = {}


def _get_nc(nblk=NBLK):
    if nblk not in _CACHED:
        nc = build_nc(nblk)
        nc.finalize()
        _CACHED[nblk] = nc
    return _CACHED[nblk]


def _prep_in_maps(q, k, v, Wq_w, Wq_b, Wk_w, Wk_b, vs_w):
    q3 = np.asarray(q, np.float32).reshape(B * H, LQ, DK)
    k3 = np.asarray(k, np.float32).reshape(B * H, LK, DK)
    v3 = np.asarray(v, np.float32).reshape(B * H, LK, DK)
    qkT = np.concatenate(
        [q3.transpose(0, 2, 1), k3.transpose(0, 2, 1)], axis=1
    )  # [B*H, 128, 256]
    par = np.zeros((2 * DK, DK + 3), np.float32)
    par[0:DK, 0:DK] = np.asarray(Wq_w, np.float32).T
    par[DK : 2 * DK, 0:DK] = np.asarray(Wk_w, np.float32).T
    par[0:DK, DK] = np.asarray(Wq_b, np.float32).reshape(DK)
    par[0:DK, DK + 1] = np.asarray(Wk_b, np.float32).reshape(DK)
    par[0:DK, DK + 2] = np.asarray(vs_w, np.float32).reshape(DK)
    in_maps = []
    for i in range(NCORES):
        s = slice(i * NBLK, (i + 1) * NBLK)
        in_maps.append(
            {
                "qkT": np.ascontiguousarray(qkT[s]),
                "v": np.ascontiguousarray(v3[s]),
                "par": par,
            }
        )
    return in_maps


def _run(inputs, trace=False):
    nc = _get_nc()
    in_maps = _prep_in_maps(
        inputs["q"], inputs["k"], inputs["v"], inputs["Wq_w"], inputs["Wq_b"],
        inputs["Wk_w"], inputs["Wk_b"], inputs["vs_w"],
    )
    res = run_bass_kernel_spmd(nc, in_maps, list(range(NCORES)), trace=trace)
    outs = np.stack([res.results[i]["out"] for i in range(NCORES)])
    attnPs = np.stack([res.results[i]["attnP"] for i in range(NCORES)])
    output = outs.reshape(B, H, LQ, DK)
    # attnP[c, b, p, half, h, ql] -> attn[q = half*128+ql, k = h*128+p]
    attn = (
        attnPs.reshape(B * H, 128, 2, 2, 128)
        .transpose(0, 2, 4, 3, 1)
        .reshape(B, H, LQ, LK)
    )
    return (output, np.ascontiguousarray(attn)), res


def kernel(q, k, v, Wq_w, Wq_b, Wk_w, Wk_b, vs_w, vs_b):
    (output, attn), _ = _run(
        {
            "q": q, "k": k, "v": v, "Wq_w": Wq_w, "Wq_b": Wq_b,
            "Wk_w": Wk_w, "Wk_b": Wk_b, "vs_w": vs_w,
        }
    )
    return output, attn


# revision 33
# speedup vs baseline: 1.2609x; 1.2292x over previous
"""Additive (Bahdanau) attention kernel for 8 Trainium2 NeuronCores.

Reference computation (per (b,h) block, Lq=Lk=256, dk=64):
    qp = q @ Wq_w.T + Wq_b
    kp = k @ Wk_w.T + Wk_b
    scores[q,k] = vs_w . tanh(qp[q,:] + kp[k,:]) + vs_b
    attn = softmax(scores, axis=k)
    out  = attn @ v
Returns (out, attn).

Key observations:
  * vs_b is constant along k, so softmax cancels it exactly -> dropped.
  * |scores| <= sum|vs_w| <= 8, so exp() needs no max-subtraction.
  * B*H = 32 independent blocks -> 4 per core, params replicated, no
    collectives.

Device layout (per block): partitions = 128 = 2 x dk, packing the query
pair (2j, 2j+1) per tile column; k runs along the free dim.
  - kp2 [128, 256]  : kp^T duplicated in both partition halves
  - qp2 [128, 128]  : column j = [qp^T[:,2j] ; qp^T[:,2j+1]]
  - DVE tensor_scalar add builds pre[:, j-slice] = kp2 + qp2[:,j]
  - ACT does one big tanh per batch of pairs (the DVE adds are the
    kernel's critical path: DVE fast modes don't engage on this build,
    so everything is sized around 1x DVE streaming)
  - PE reduces over dk with stationary=tanh tile (bf16, FWL) and
    moving=vs2 [128,2] block-diagonal vs_w -> scoresT [k, q] in PSUM,
    one PSUM tile per q-half so softmax can drain early
  - softmax over k (partitions) via ones-matmul column sums, chunked by
    q-half to shorten the kernel tail
  - out = attnT.T @ v needs no transpose since attnT already has k on
    partitions; the host transposes attnT -> attn at the end.
"""

import os
from contextlib import ExitStack

import numpy as np

os.environ.setdefault("MYCRO_LOCAL_CACHE", "1")

import concourse.bass as bass
import concourse.bacc as bacc
import concourse.tile as tile
from concourse import mybir
from concourse.bass_utils import run_bass_kernel_spmd

F32 = mybir.dt.float32
BF16 = mybir.dt.bfloat16
AF = mybir.ActivationFunctionType

B, H, LQ, LK, DK = 4, 8, 256, 256, 64
NCORES = 8
NBLK = (B * H) // NCORES  # blocks per core
NPAIR = LQ // 2  # q-pairs per block; pair j = queries (2j, 2j+1)
PAIRS_PER_HALF = NPAIR // 2  # pairs per q-half (queries 0:128 / 128:256)


def batches_for_block(b, nblk):
    if b < nblk - 1:
        return [16] * 8
    # taper the final block so the ACT pipeline drains quickly
    return [16] * 6 + [8] * 4


def build_nc(nblk=NBLK):
    nc = bacc.Bacc(None, target_bir_lowering=False)
    # qkT[b]: [128, 256] = [qT (d x Lq) ; kT (d x Lk)] stacked on partitions
    qkT_d = nc.declare_dram_parameter("qkT", [nblk, 2 * DK, LQ], F32, isOutput=False)
    v_d = nc.declare_dram_parameter("v", [nblk, LK, DK], F32, isOutput=False)
    # par[0:64, 0:64]=WqT, par[64:128, 0:64]=WkT (each projection matmul then
    # has lhsT/rhs at the same base partition); par[0:64, 64]=Wqb,
    # par[0:64, 65]=Wkb, par[0:64, 66]=vs
    par_d = nc.declare_dram_parameter("par", [2 * DK, DK + 3], F32, isOutput=False)
    eye_d = nc.declare_dram_parameter("eye", [128, 128], F32, isOutput=False)
    out_d = nc.declare_dram_parameter("out", [nblk, LQ, DK], F32, isOutput=True)
    # attnP[b, p, half, h, ql] = attn^T[b, k=h*128+p, q=half*128+ql] — laid out
    # so each q-half DMA writes 1KB contiguous per partition; host unscrambles.
    attnP_d = nc.declare_dram_parameter(
        "attnP", [nblk, 128, 2, 2, 128], F32, isOutput=True
    )

    with ExitStack() as ctx:
        tc = ctx.enter_context(tile.TileContext(nc))
        consts = ctx.enter_context(tc.tile_pool(name="consts", bufs=1))
        proj_in = ctx.enter_context(tc.tile_pool(name="proj_in", bufs=2))
        blk = ctx.enter_context(tc.tile_pool(name="blk", bufs=2))
        pre_pool = ctx.enter_context(tc.tile_pool(name="pre", bufs=3))
        tanh_pool = ctx.enter_context(tc.tile_pool(name="tanh", bufs=3))
        soft = ctx.enter_context(tc.tile_pool(name="soft", bufs=2))
        ps_proj = ctx.enter_context(tc.tile_pool(name="ps_proj", bufs=2, space="PSUM"))
        ps_scores = ctx.enter_context(
            tc.tile_pool(name="ps_scores", bufs=2, space="PSUM")
        )
        ps_small = ctx.enter_context(tc.tile_pool(name="ps_small", bufs=2, space="PSUM"))

        # ---- constants (single DMA) ----
        par_sb = consts.tile([2 * DK, DK + 3], F32)
        nc.sync.dma_start(out=par_sb, in_=par_d[:, :])
        WqT_sb = par_sb[0:DK, 0:DK]
        WkT_sb = par_sb[DK : 2 * DK, 0:DK]
        Wqb_sb = par_sb[0:DK, DK : DK + 1]
        Wkb_sb = par_sb[0:DK, DK + 1 : DK + 2]
        vsc_sb = par_sb[0:DK, DK + 2 : DK + 3]

        # vs2 [128, 2] block diagonal: col0 = [vs;0], col1 = [0;vs]
        vs2 = consts.tile([128, 2], BF16)
        nc.vector.memset(vs2, 0.0)
        nc.vector.tensor_copy(vs2[0:DK, 0:1], vsc_sb)
        nc.vector.tensor_copy(vs2[DK : 2 * DK, 1:2], vsc_sb)

        ones_m = consts.tile([1, 128], BF16)  # lhsT for recip broadcast (K=1)
        nc.vector.memset(ones_m, 1.0)
        ones_k = consts.tile([128, 1], BF16)  # rhs for per-partition sums (N=1)
        nc.vector.memset(ones_k, 1.0)
        eye_sb = consts.tile([128, 128], F32)  # identity for PE transpose
        nc.scalar.dma_start(out=eye_sb, in_=eye_d[:, :])

        state = {}

        def prologue(b):
            """DMA + projections + pair packing for block b. Input DMAs ride
            the ACT HWDGE queue so they don't contend with output DMAs."""
            qkT_sb = proj_in.tile([2 * DK, LQ], F32, tag="qkT_sb")
            nc.scalar.dma_start(out=qkT_sb, in_=qkT_d[b])
            v_sb = proj_in.tile([128, 2, DK], F32, tag="v_sb")
            nc.scalar.dma_start(
                out=v_sb, in_=v_d[b].rearrange("(h p) d -> p h d", p=128)
            )

            # split even/odd q projections so the pair-packing reads psum
            # contiguously
            ps_qe = ps_proj.tile([DK, NPAIR], F32, tag="ps_proj")
            nc.tensor.matmul(
                ps_qe, lhsT=WqT_sb, rhs=qkT_sb[0:DK, 0:LQ:2], start=True, stop=True
            )
            ps_qo = ps_proj.tile([DK, NPAIR], F32, tag="ps_proj")
            nc.tensor.matmul(
                ps_qo, lhsT=WqT_sb, rhs=qkT_sb[0:DK, 1:LQ:2], start=True, stop=True
            )
            qp2 = blk.tile([128, NPAIR], F32, tag="qp2")
            nc.vector.tensor_scalar_add(qp2[0:DK, :], ps_qe, Wqb_sb)
            nc.vector.tensor_scalar_add(qp2[DK : 2 * DK, :], ps_qo, Wqb_sb)

            ps_kp = ps_proj.tile([DK, LK], F32, tag="ps_proj")
            nc.tensor.matmul(
                ps_kp, lhsT=WkT_sb, rhs=qkT_sb[DK : 2 * DK, :], start=True, stop=True
            )
            kp2 = blk.tile([128, LK], BF16, tag="kp2")
            nc.vector.tensor_scalar_add(kp2[0:DK, :], ps_kp, Wkb_sb)
            nc.vector.tensor_scalar_add(kp2[DK : 2 * DK, :], ps_kp, Wkb_sb)
            v_bf = blk.tile([128, 2, DK], BF16, tag="v_bf")
            nc.vector.tensor_copy(v_bf, v_sb)
            state[b] = (qp2, kp2, v_bf)

        def main_loop(b):
            """DVE adds + ACT tanh + PE score reductions for block b."""
            qp2, kp2, v_sb = state[b]
            # scoresT psum per q-half: cols = h*128 + q_local
            ps_a = ps_scores.tile([128, LQ], F32, tag="ps_a")
            ps_b = ps_scores.tile([128, LQ], F32, tag="ps_b")

            state[b] = (qp2, kp2, v_sb, ps_a, ps_b)
            attnT_f = soft.tile([128, 2, 2, 128], F32, tag="attnT_f")

            j = 0
            for nb in batches_for_block(b, nblk):
                pre = pre_pool.tile([128, 16 * LK], BF16, tag="pre")
                th = tanh_pool.tile([128, 16 * LK], BF16, tag="tanh")
                for jj in range(nb):
                    nc.vector.tensor_scalar_add(
                        pre[:, jj * LK : (jj + 1) * LK], kp2, qp2[:, j + jj : j + jj + 1]
                    )
                nc.scalar.activation(
                    th[:, 0 : nb * LK], pre[:, 0 : nb * LK], AF.Tanh
                )
                for jj in range(nb):
                    jt = j + jj
                    half, jl = divmod(jt, PAIRS_PER_HALF)
                    ps_half = ps_a if half == 0 else ps_b
                    for h in range(2):
                        nc.tensor.matmul(
                            ps_half[:, h * 128 + 2 * jl : h * 128 + 2 * jl + 2],
                            lhsT=th[:, jj * LK + h * 128 : jj * LK + (h + 1) * 128],
                            rhs=vs2,
                            start=True,
                            stop=True,
                        )
                j += nb
                if j == PAIRS_PER_HALF:
                    # q-half 0 is complete: drain its softmax now so only
                    # half 1's chain sits on the kernel tail
                    softmax_half(b, 0, attnT_f)
            softmax_half(b, 1, attnT_f)

        def softmax_half(b, half, attnT_f):
            """exp -> sums -> normalize -> attn@v for one q-half of block b.

            Sums land per-partition ([128,1] via lhsT=exp) so reciprocal runs
            on all lanes; the output path scales unnormalized attn@v rows by
            recip; the attn output gets recip broadcast via PE transpose."""
            _, _, v_sb, ps_a, ps_b = state[b]
            ps_half = ps_a if half == 0 else ps_b
            exp_sb = soft.tile([128, LQ], BF16, tag="exp_sb")
            nc.scalar.activation(exp_sb, ps_half, AF.Exp)
            # sums[q] = sum_k expT[k, q]: lhsT=exp slice, rhs=ones -> [128, 1]
            ps_sum = ps_small.tile([128, 1], F32, tag="ps_small")
            for h in range(2):
                nc.tensor.matmul(
                    ps_sum,
                    lhsT=exp_sb[:, h * 128 : (h + 1) * 128],
                    rhs=ones_k,
                    start=(h == 0),
                    stop=(h == 1),
                )
            recip_f = soft.tile([128, 1], F32, tag="recip_f")
            nc.vector.reciprocal(recip_f, ps_sum)

            # unnormalized attn@v, then scale rows by recip (free normalize)
            ps_o = ps_small.tile([128, DK], F32, tag="ps_small")
            for h in range(2):
                nc.tensor.matmul(
                    ps_o,
                    lhsT=exp_sb[:, h * 128 : (h + 1) * 128],
                    rhs=v_sb[:, h, :],
                    start=(h == 0),
                    stop=(h == 1),
                )
            o_sb = soft.tile([128, DK], F32, tag="o_sb")
            nc.vector.tensor_scalar_mul(o_sb, ps_o, recip_f)
            nc.sync.dma_start(
                out=out_d[b, half * 128 : (half + 1) * 128, :], in_=o_sb
            )

            # attn output: recip -> row via PE transpose -> broadcast -> mul
            ps_row = ps_small.tile([1, 128], F32, tag="ps_small")
            nc.tensor.transpose(ps_row, recip_f, eye_sb)
            row_bf = soft.tile([1, 128], BF16, tag="row_bf")
            nc.vector.tensor_copy(row_bf, ps_row)
            ps_bc = ps_small.tile([128, 128], F32, tag="ps_small")
            nc.tensor.matmul(ps_bc, lhsT=ones_m, rhs=row_bf, start=True, stop=True)
            for h in range(2):
                nc.vector.tensor_mul(
                    attnT_f[:, half, h, :],
                    exp_sb[:, h * 128 : (h + 1) * 128],
                    ps_bc,
                )
            nc.sync.dma_start(out=attnP_d[b, :, half], in_=attnT_f[:, half])

        prologue(0)
        for b in range(nblk):
            main_loop(b)
            if b + 1 < nblk:
                prologue(b + 1)
            del state[b]

    return nc


_CACHED = {}


def _get_nc(nblk=NBLK):
    if nblk not in _CACHED:
        nc = build_nc(nblk)
        nc.finalize()
        _CACHED[nblk] = nc
    return _CACHED[nblk]


def _prep_in_maps(q, k, v, Wq_w, Wq_b, Wk_w, Wk_b, vs_w):
    q3 = np.asarray(q, np.float32).reshape(B * H, LQ, DK)
    k3 = np.asarray(k, np.float32).reshape(B * H, LK, DK)
    v3 = np.asarray(v, np.float32).reshape(B * H, LK, DK)
    qkT = np.concatenate(
        [q3.transpose(0, 2, 1), k3.transpose(0, 2, 1)], axis=1
    )  # [B*H, 128, 256]
    par = np.zeros((2 * DK, DK + 3), np.float32)
    par[0:DK, 0:DK] = np.asarray(Wq_w, np.float32).T
    par[DK : 2 * DK, 0:DK] = np.asarray(Wk_w, np.float32).T
    par[0:DK, DK] = np.asarray(Wq_b, np.float32).reshape(DK)
    par[0:DK, DK + 1] = np.asarray(Wk_b, np.float32).reshape(DK)
    par[0:DK, DK + 2] = np.asarray(vs_w, np.float32).reshape(DK)
    eye = np.eye(128, dtype=np.float32)
    in_maps = []
    for i in range(NCORES):
        s = slice(i * NBLK, (i + 1) * NBLK)
        in_maps.append(
            {
                "qkT": np.ascontiguousarray(qkT[s]),
                "v": np.ascontiguousarray(v3[s]),
                "par": par,
                "eye": eye,
            }
        )
    return in_maps


def _run(inputs, trace=False):
    nc = _get_nc()
    in_maps = _prep_in_maps(
        inputs["q"], inputs["k"], inputs["v"], inputs["Wq_w"], inputs["Wq_b"],
        inputs["Wk_w"], inputs["Wk_b"], inputs["vs_w"],
    )
    res = run_bass_kernel_spmd(nc, in_maps, list(range(NCORES)), trace=trace)
    outs = np.stack([res.results[i]["out"] for i in range(NCORES)])
    attnPs = np.stack([res.results[i]["attnP"] for i in range(NCORES)])
    output = outs.reshape(B, H, LQ, DK)
    # attnP[c, b, p, half, h, ql] -> attn[q = half*128+ql, k = h*128+p]
    attn = (
        attnPs.reshape(B * H, 128, 2, 2, 128)
        .transpose(0, 2, 4, 3, 1)
        .reshape(B, H, LQ, LK)
    )
    return (output, np.ascontiguousarray(attn)), res


def kernel(q, k, v, Wq_w, Wq_b, Wk_w, Wk_b, vs_w, vs_b):
    (output, attn), _ = _run(
        {
            "q": q, "k": k, "v": v, "Wq_w": Wq_w, "Wq_b": Wq_b,
            "Wk_w": Wk_w, "Wk_b": Wk_b, "vs_w": vs_w,
        }
    )
    return output, attn
